# revision 17
# baseline (speedup 1.0000x reference)
"""Fused QKV + multi-head attention kernel for Trainium2 (Bass/Tile), 8-core SPMD.

Problem: x[4, 2048, 1024] -> qkv = x @ W_qkv + b_qkv -> 16-head attention -> out[4, 2048, 1024].

Sharding (DP x TP): core c handles batch c//2 and head-group c%2 (8 of 16 heads),
so each core runs the qkv projection for its batch restricted to its heads'
columns of W_qkv, plus full attention for its 8 heads. No cross-core comm.

Per-core kernel design (bf16 matmuls, fp32 accumulation):
 - the host pre-transposes x during sharding, so the device loads xT[k, tok]
   directly with a casting DMA (f32->bf16); no on-device transposes anywhere.
 - qk projection computes qkv^T directly: lhsT = W tile (layout [k, feat] as
   stored), rhs = xT. Heads are processed in pairs: head 2p lives in SBUF
   partitions 0-63 and head 2p+1 in partitions 64-127 (both q^T and k^T), which
   makes the K=64 score matmuls legal (lhsT/rhs share a base partition) and
   lets the PE run the two heads on independent 64-row array tiles.
 - v projection computes v in [tok, feat] orientation (lhsT = xT stationary,
   rhs = W v-columns), with a ones-column appended per head so the attention
   A@V matmul also produces the softmax denominator.
 - scores^T = k^T.T @ q^T accumulate in PSUM; exp (with the 1/8 scale folded
   into the ACT affine) reads [128, 2048] PSUM spans to amortize ACT overhead;
   no max-subtraction (scores ~ N(0,1), exp never overflows fp32/bf16).
 - out^T[65, 512] = [v | 1].T @ E accumulates over key tiles; the denominator
   row is bounced through DRAM into a [128, 4] layout for a 128-lane reciprocal,
   broadcast back across partitions with a stride-0 DMA, and the normalized
   out^T (+ v-bias, added after normalization -- exact) is stored transposed;
   the host un-transposes during unsharding.
"""

import sys

sys.path.insert(0, "/opt/trn_rl_repo")

import numpy as np

T = 2048
D = 1024
NH_LOCAL = 8  # heads per core
HS = 64
WCOLS = NH_LOCAL * 3 * HS  # 1536
VCOLS = NH_LOCAL * HS  # 512
KT = D // 128  # 8 contraction tiles
PAIRS = NH_LOCAL // 2  # 4
IG = T // 512  # 4 query groups
JT = T // 128  # 16 key tiles

_CACHE: dict = {}


def _emit(tc, x_d, w_d, bqk_d, bv_d, out_d):
    import concourse.bass as bass
    from concourse import mybir
    from contextlib import ExitStack

    nc = tc.nc
    f32 = mybir.dt.float32
    bf16 = mybir.dt.bfloat16
    Exp = mybir.ActivationFunctionType.Exp

    ctx = ExitStack()
    pers = ctx.enter_context(tc.tile_pool(name="pers", bufs=1))
    qk_pool = ctx.enter_context(tc.tile_pool(name="qk", bufs=PAIRS))
    e_pool = ctx.enter_context(tc.tile_pool(name="epool", bufs=3))
    o_pool = ctx.enter_context(tc.tile_pool(name="opool", bufs=3))
    sm_pool = ctx.enter_context(tc.tile_pool(name="smpool", bufs=4))
    ot_pool = ctx.enter_context(tc.tile_pool(name="otpool", bufs=4))
    dr_pool = ctx.enter_context(tc.tile_pool(name="drpool", bufs=4, space="DRAM"))
    # PSUM: 8 banks total = ps 2 (proj) + pss 2x2 (scores/exp staging) + po 2 (AV accum).
    # pss double-buffering is what keeps the scores->exp->AV pipeline from
    # serializing (a single buffer stalls the PE ~2.7us per exp group, which
    # also re-throttles the HAM clock gate to 1.2 GHz).
    ps_pool = ctx.enter_context(tc.tile_pool(name="pspool", bufs=2, space="PSUM"))
    pss_pool = ctx.enter_context(tc.tile_pool(name="psspool", bufs=2, space="PSUM"))
    po_pool = ctx.enter_context(tc.tile_pool(name="popool", bufs=2, space="PSUM"))

    # ---- load + cast xT (host pre-transposes x, so this is a plain cast DMA) ----
    xt_sb = pers.tile([128, KT, T], bf16)  # xT[k, tok] per k-tile
    for kk in range(KT):
        nc.gpsimd.dma_start(xt_sb[:, kk, :], x_d[kk * 128 : (kk + 1) * 128, :])

    # ---- constants ----
    bqk_sb = pers.tile([128, PAIRS, 2], f32)  # [part, pair, q/k] per-partition bias
    nc.sync.dma_start(bqk_sb, bqk_d)
    bv_pp = pers.tile([64, NH_LOCAL], f32)  # v-bias, per-partition layout [d, head]
    nc.sync.dma_start(bv_pp, bv_d.rearrange("(h d) -> d h", d=HS))

    # ---- load + cast W ----
    w_sb = pers.tile([128, KT, WCOLS], bf16)
    for kk in range(KT):
        nc.gpsimd.dma_start(w_sb[:, kk, :], w_d[kk * 128 : (kk + 1) * 128, :])  # f32->bf16 cast

    # ---- v ones-columns ----
    v_sb = pers.tile([128, JT, NH_LOCAL, HS + 1], bf16)
    nc.vector.memset(v_sb[:, :, :, HS : HS + 1], 1.0)

    # w is host-permuted: cols [(p*2+qk)*128 : +128] = paired q/k block for pair p,
    # cols [1024:1536] = v columns in head order (walrus requires single-free-dim
    # matmul operand APs, so the pairing permutation happens host-side).
    def emit_qk_proj(p, qk_t):
        # psum partitions 0-63 <- head 2p, 64-127 <- head 2p+1
        for qk in range(2):
            for g in range(IG):
                ps = ps_pool.tile([128, 512], f32, tag="ps")
                c0 = (p * 2 + qk) * 128
                for kk in range(KT):
                    nc.tensor.matmul(
                        ps,
                        w_sb[:, kk, c0 : c0 + 128],
                        xt_sb[:, kk, g * 512 : (g + 1) * 512],
                        start=(kk == 0),
                        stop=(kk == KT - 1),
                    )
                nc.vector.tensor_scalar_add(
                    qk_t[:, qk, g * 512 : (g + 1) * 512], ps, bqk_sb[:, p, qk : qk + 1]
                )

    def emit_v_proj():
        for tt in range(JT):
            ps = ps_pool.tile([128, 512], f32, tag="ps")
            for kk in range(KT):
                nc.tensor.matmul(
                    ps,
                    xt_sb[:, kk, tt * 128 : (tt + 1) * 128],
                    w_sb[:, kk, 1024:1536],
                    start=(kk == 0),
                    stop=(kk == KT - 1),
                )
            nc.vector.tensor_copy(
                v_sb[:, tt, :, 0:HS], ps.rearrange("p (h c) -> p h c", c=HS)
            )

    def emit_attention(p, qk_t):
        for ig in range(IG):
            po = [po_pool.tile([65, 512], f32, tag="po", name=f"po{_h}") for _h in range(2)]
            for jj in range(JT):
                ps = pss_pool.tile([128, 1024], f32, tag="pss")
                for h in range(2):
                    base = 64 * h
                    nc.tensor.matmul(
                        ps[:, h * 512 : (h + 1) * 512],
                        qk_t[base : base + 64, 1, jj * 128 : (jj + 1) * 128],
                        qk_t[base : base + 64, 0, ig * 512 : (ig + 1) * 512],
                        start=True,
                        stop=True,
                    )
                e_t = e_pool.tile([128, 1024], bf16, tag="e")
                nc.scalar.activation(e_t, ps, Exp, scale=0.125)
                for h in range(2):
                    nc.tensor.matmul(
                        po[h],
                        v_sb[:, jj, 2 * p + h, :],
                        e_t[:, h * 512 : (h + 1) * 512],
                        start=(jj == 0),
                        stop=(jj == JT - 1),
                    )
            # normalize: out^T[d, i] = po[d, i] * (1/den[i]) + bv[d], written as out^T.
            # den row sits on partition 64; the reciprocal is computed across 128
            # lanes by bouncing the row through DRAM into a [128, 4] layout, and
            # broadcast back across partitions with a stride-0 DMA.
            o_t = [o_pool.tile([65, 512], f32, tag="o", name=f"ot{_h}") for _h in range(2)]
            den_t = dr_pool.tile([2, 512], f32, tag="dend")
            rcd_t = dr_pool.tile([2, 512], f32, tag="rcd")
            for h in range(2):
                nc.vector.tensor_copy(o_t[h], po[h])
                nc.sync.dma_start(den_t[h], o_t[h][64:65, :])
            rct = sm_pool.tile([128, 8], f32, tag="rct")
            den_src = bass.AP(
                tensor=den_t.tensor,
                offset=den_t.offset,
                ap=[[4, 128], [512, 2], [1, 4]],
            )
            nc.sync.dma_start(rct, den_src)
            rcp = sm_pool.tile([128, 8], f32, tag="rcp")
            nc.vector.reciprocal(rcp, rct)
            for h in range(2):
                nc.sync.dma_start(rcd_t[h], rcp[:, h * 4 : (h + 1) * 4])
            for h in range(2):
                den_bc = sm_pool.tile([64, 512], f32, tag="denbc", name=f"dbc{h}")
                rcd_h = rcd_t[h]
                bc_src = bass.AP(
                    tensor=rcd_h.tensor,
                    offset=rcd_h.offset,
                    ap=[[0, 64]] + list(rcd_h.ap),
                )
                nc.gpsimd.dma_start(den_bc, bc_src)
                ot = ot_pool.tile([64, 512], f32, tag="ot")
                nc.vector.tensor_mul(ot, o_t[h][0:64, :], den_bc)
                nc.vector.tensor_scalar_add(
                    ot, ot, bv_pp[:, 2 * p + h : 2 * p + h + 1]
                )
                nc.sync.dma_start(
                    out_d[(2 * p + h) * HS : (2 * p + h + 1) * HS, ig * 512 : (ig + 1) * 512],
                    ot,
                )

    qk_tiles = [qk_pool.tile([128, 2, T], bf16, tag="qkt", name=f"qkt{_p}") for _p in range(PAIRS)]
    emit_qk_proj(0, qk_tiles[0])
    emit_v_proj()
    for p in range(PAIRS):
        if p + 1 < PAIRS:
            emit_qk_proj(p + 1, qk_tiles[p + 1])
        emit_attention(p, qk_tiles[p])
    ctx.close()


def _build():
    import concourse.tile as tile
    from concourse import bacc, mybir

    f32 = mybir.dt.float32
    nc = bacc.Bacc("TRN2", target_bir_lowering=False, debug=False, num_devices=8)
    x_d = nc.dram_tensor("x", [D, T], f32, kind="ExternalInput").ap()
    w_d = nc.dram_tensor("w", [D, WCOLS], f32, kind="ExternalInput").ap()
    bqk_d = nc.dram_tensor("bqk", [128, PAIRS, 2], f32, kind="ExternalInput").ap()
    bv_d = nc.dram_tensor("bv", [VCOLS], f32, kind="ExternalInput").ap()
    out_d = nc.dram_tensor("out", [VCOLS, T], f32, kind="ExternalOutput").ap()
    with tile.TileContext(nc) as tc:
        _emit(tc, x_d, w_d, bqk_d, bv_d, out_d)
    nc.compile()
    return nc


def get_nc():
    if "nc" not in _CACHE:
        _CACHE["nc"] = _build()
    return _CACHE["nc"]


def make_in_maps(x, W_qkv, b_qkv):
    """Shard full inputs into 8 per-core input maps."""
    x = np.asarray(x, dtype=np.float32)
    W_qkv = np.asarray(W_qkv, dtype=np.float32)
    b_qkv = np.asarray(b_qkv, dtype=np.float32)
    in_maps = []
    for c in range(8):
        b, half = divmod(c, 2)
        w_c = W_qkv[:, half * WCOLS : (half + 1) * WCOLS]
        b_c = b_qkv[half * WCOLS : (half + 1) * WCOLS]
        # permute columns: paired q/k blocks first, then v cols in head order
        w3 = w_c.reshape(D, NH_LOCAL, 3, HS)
        blocks = []
        for p in range(PAIRS):
            for qk in range(2):
                blocks.append(w3[:, 2 * p, qk, :])
                blocks.append(w3[:, 2 * p + 1, qk, :])
        for h in range(NH_LOCAL):
            blocks.append(w3[:, h, 2, :])
        w_c = np.ascontiguousarray(np.concatenate(blocks, axis=1))
        # per-partition qk bias: partitions 0-63 <- head 2p, 64-127 <- head 2p+1
        bqk = np.zeros((128, PAIRS, 2), dtype=np.float32)
        for p in range(PAIRS):
            for qk in range(2):
                bqk[0:64, p, qk] = b_c[(2 * p) * 192 + qk * 64 : (2 * p) * 192 + (qk + 1) * 64]
                bqk[64:128, p, qk] = b_c[(2 * p + 1) * 192 + qk * 64 : (2 * p + 1) * 192 + (qk + 1) * 64]
        bv = np.ascontiguousarray(
            b_c.reshape(NH_LOCAL, 3, HS)[:, 2, :].reshape(VCOLS)
        )
        in_maps.append(
            {
                "x": np.ascontiguousarray(x[b].T),
                "w": w_c,
                "bqk": bqk,
                "bv": bv,
            }
        )
    return in_maps


def assemble_output(results):
    out = np.zeros((4, T, D), dtype=np.float32)
    for c in range(8):
        b, half = divmod(c, 2)
        out[b, :, half * VCOLS : (half + 1) * VCOLS] = results[c]["out"].T
    return out


def kernel(x, W_qkv, b_qkv):
    from concourse.bass_utils import run_bass_kernel_spmd

    nc = get_nc()
    in_maps = make_in_maps(x, W_qkv, b_qkv)
    res = run_bass_kernel_spmd(nc, in_maps, core_ids=list(range(8)))
    return assemble_output(res.results)


if __name__ == "__main__":
    xs = np.random.randn(4, T, D).astype(np.float32)
    Ws = (np.random.randn(D, 3 * D) / 32.0).astype(np.float32)
    bs = (np.random.randn(3 * D) * 0.02).astype(np.float32)
    o = kernel(xs, Ws, bs)
    print(o.shape, o.dtype)


# revision 19
# speedup vs baseline: 1.0682x; 1.0682x over previous
"""Fused QKV + multi-head attention kernel for Trainium2 (Bass/Tile), 8-core SPMD.

Problem: x[4, 2048, 1024] -> qkv = x @ W_qkv + b_qkv -> 16-head attention -> out[4, 2048, 1024].

Sharding (DP x TP): core c handles batch c//2 and head-group c%2 (8 of 16 heads),
so each core runs the qkv projection for its batch restricted to its heads'
columns of W_qkv, plus full attention for its 8 heads. No cross-core comm.

Per-core kernel design (bf16 matmuls, fp32 accumulation):
 - the host pre-transposes x during sharding, so the device loads xT[k, tok]
   directly with a casting DMA (f32->bf16); no on-device transposes anywhere.
 - qk projection computes qkv^T directly: lhsT = W tile (layout [k, feat] as
   stored), rhs = xT. Heads are processed in pairs: head 2p lives in SBUF
   partitions 0-63 and head 2p+1 in partitions 64-127 (both q^T and k^T), which
   makes the K=64 score matmuls legal (lhsT/rhs share a base partition) and
   lets the PE run the two heads on independent 64-row array tiles.
 - v projection computes v in [tok, feat] orientation (lhsT = xT stationary,
   rhs = W v-columns), with a ones-column appended per head so the attention
   A@V matmul also produces the softmax denominator.
 - scores^T = k^T.T @ q^T accumulate in PSUM; exp (with the 1/8 scale folded
   into the ACT affine) reads [128, 2048] PSUM spans to amortize ACT overhead;
   no max-subtraction (scores ~ N(0,1), exp never overflows fp32/bf16).
 - out^T[65, 512] = [v | 1].T @ E accumulates over key tiles; the denominator
   row is bounced through DRAM into a [128, 4] layout for a 128-lane reciprocal,
   broadcast back across partitions with a stride-0 DMA, and the normalized
   out^T (+ v-bias, added after normalization -- exact) is stored transposed;
   the host un-transposes during unsharding.
"""

import sys

sys.path.insert(0, "/opt/trn_rl_repo")

import numpy as np
import ml_dtypes

T = 2048
D = 1024
NH_LOCAL = 8  # heads per core
HS = 64
WCOLS = NH_LOCAL * 3 * HS  # 1536
VCOLS = NH_LOCAL * HS  # 512
KT = D // 128  # 8 contraction tiles
PAIRS = NH_LOCAL // 2  # 4
IG = T // 512  # 4 query groups
JT = T // 128  # 16 key tiles

_CACHE: dict = {}


def _emit(tc, x_d, w_d, bqk_d, bv_d, out_d):
    import concourse.bass as bass
    from concourse import mybir
    from contextlib import ExitStack

    nc = tc.nc
    f32 = mybir.dt.float32
    bf16 = mybir.dt.bfloat16
    Exp = mybir.ActivationFunctionType.Exp

    ctx = ExitStack()
    pers = ctx.enter_context(tc.tile_pool(name="pers", bufs=1))
    qk_pool = ctx.enter_context(tc.tile_pool(name="qk", bufs=PAIRS))
    e_pool = ctx.enter_context(tc.tile_pool(name="epool", bufs=3))
    o_pool = ctx.enter_context(tc.tile_pool(name="opool", bufs=3))
    sm_pool = ctx.enter_context(tc.tile_pool(name="smpool", bufs=4))
    ot_pool = ctx.enter_context(tc.tile_pool(name="otpool", bufs=4))
    dr_pool = ctx.enter_context(tc.tile_pool(name="drpool", bufs=4, space="DRAM"))
    # PSUM: 8 banks total = ps 2 (proj) + pss 2x2 (scores/exp staging) + po 2 (AV accum).
    # pss double-buffering is what keeps the scores->exp->AV pipeline from
    # serializing (a single buffer stalls the PE ~2.7us per exp group, which
    # also re-throttles the HAM clock gate to 1.2 GHz).
    ps_pool = ctx.enter_context(tc.tile_pool(name="pspool", bufs=2, space="PSUM"))
    pss_pool = ctx.enter_context(tc.tile_pool(name="psspool", bufs=2, space="PSUM"))
    po_pool = ctx.enter_context(tc.tile_pool(name="popool", bufs=2, space="PSUM"))

    # ---- load xT (host pre-transposes and pre-casts to bf16) ----
    xt_sb = pers.tile([128, KT, T], bf16)  # xT[k, tok] per k-tile
    for kk in range(KT):
        nc.sync.dma_start(xt_sb[:, kk, :], x_d[kk * 128 : (kk + 1) * 128, :])

    # ---- constants ----
    bqk_sb = pers.tile([128, PAIRS, 2], f32)  # [part, pair, q/k] per-partition bias
    nc.sync.dma_start(bqk_sb, bqk_d)
    bv_pp = pers.tile([64, NH_LOCAL], f32)  # v-bias, per-partition layout [d, head]
    nc.sync.dma_start(bv_pp, bv_d.rearrange("(h d) -> d h", d=HS))

    # ---- load W (host pre-casts to bf16) ----
    w_sb = pers.tile([128, KT, WCOLS], bf16)
    for kk in range(KT):
        nc.sync.dma_start(w_sb[:, kk, :], w_d[kk * 128 : (kk + 1) * 128, :])

    # ---- v ones-columns ----
    v_sb = pers.tile([128, JT, NH_LOCAL, HS + 1], bf16)
    nc.vector.memset(v_sb[:, :, :, HS : HS + 1], 1.0)

    # w is host-permuted: cols [(p*2+qk)*128 : +128] = paired q/k block for pair p,
    # cols [1024:1536] = v columns in head order (walrus requires single-free-dim
    # matmul operand APs, so the pairing permutation happens host-side).
    def emit_qk_proj(p, qk_t):
        # psum partitions 0-63 <- head 2p, 64-127 <- head 2p+1
        for g in range(IG):
            for qk in range(2):
                ps = ps_pool.tile([128, 512], f32, tag="ps")
                c0 = (p * 2 + qk) * 128
                for kk in range(KT):
                    nc.tensor.matmul(
                        ps,
                        w_sb[:, kk, c0 : c0 + 128],
                        xt_sb[:, kk, g * 512 : (g + 1) * 512],
                        start=(kk == 0),
                        stop=(kk == KT - 1),
                    )
                nc.vector.tensor_scalar_add(
                    qk_t[:, qk, g * 512 : (g + 1) * 512], ps, bqk_sb[:, p, qk : qk + 1]
                )

    def emit_v_proj(p):
        for tt in range(JT):
            ps = ps_pool.tile([128, 512], f32, tag="ps")
            for kk in range(KT):
                nc.tensor.matmul(
                    ps[:, 0:128],
                    xt_sb[:, kk, tt * 128 : (tt + 1) * 128],
                    w_sb[:, kk, 1024 + p * 128 : 1024 + (p + 1) * 128],
                    start=(kk == 0),
                    stop=(kk == KT - 1),
                )
            nc.vector.tensor_copy(
                v_sb[:, tt, 2 * p : 2 * p + 2, 0:HS],
                ps[:, 0:128].rearrange("p (h c) -> p h c", c=HS),
            )

    def emit_attention(p, qk_t):
        for ig in range(IG):
            po = [po_pool.tile([65, 512], f32, tag="po", name=f"po{_h}") for _h in range(2)]
            for jj in range(JT):
                ps = pss_pool.tile([128, 1024], f32, tag="pss")
                for h in range(2):
                    base = 64 * h
                    nc.tensor.matmul(
                        ps[:, h * 512 : (h + 1) * 512],
                        qk_t[base : base + 64, 1, jj * 128 : (jj + 1) * 128],
                        qk_t[base : base + 64, 0, ig * 512 : (ig + 1) * 512],
                        start=True,
                        stop=True,
                    )
                e_t = e_pool.tile([128, 1024], bf16, tag="e")
                nc.scalar.activation(e_t, ps, Exp, scale=0.125)
                for h in range(2):
                    nc.tensor.matmul(
                        po[h],
                        v_sb[:, jj, 2 * p + h, :],
                        e_t[:, h * 512 : (h + 1) * 512],
                        start=(jj == 0),
                        stop=(jj == JT - 1),
                    )
            # normalize: out^T[d, i] = po[d, i] * (1/den[i]) + bv[d], written as out^T.
            # den row sits on partition 64; the reciprocal is computed across 128
            # lanes by bouncing the row through DRAM into a [128, 4] layout, and
            # broadcast back across partitions with a stride-0 DMA.
            o_t = [o_pool.tile([65, 512], f32, tag="o", name=f"ot{_h}") for _h in range(2)]
            den_t = dr_pool.tile([2, 512], f32, tag="dend")
            rcd_t = dr_pool.tile([2, 512], f32, tag="rcd")
            for h in range(2):
                nc.vector.tensor_copy(o_t[h], po[h])
                nc.sync.dma_start(den_t[h], o_t[h][64:65, :])
            rct = sm_pool.tile([128, 8], f32, tag="rct")
            den_src = bass.AP(
                tensor=den_t.tensor,
                offset=den_t.offset,
                ap=[[4, 128], [512, 2], [1, 4]],
            )
            nc.sync.dma_start(rct, den_src)
            rcp = sm_pool.tile([128, 8], f32, tag="rcp")
            nc.vector.reciprocal(rcp, rct)
            for h in range(2):
                nc.sync.dma_start(rcd_t[h], rcp[:, h * 4 : (h + 1) * 4])
            for h in range(2):
                den_bc = sm_pool.tile([64, 512], f32, tag="denbc", name=f"dbc{h}")
                rcd_h = rcd_t[h]
                bc_src = bass.AP(
                    tensor=rcd_h.tensor,
                    offset=rcd_h.offset,
                    ap=[[0, 64]] + list(rcd_h.ap),
                )
                nc.gpsimd.dma_start(den_bc, bc_src)
                ot = ot_pool.tile([64, 512], f32, tag="ot")
                nc.vector.tensor_mul(ot, o_t[h][0:64, :], den_bc)
                nc.vector.tensor_scalar_add(
                    ot, ot, bv_pp[:, 2 * p + h : 2 * p + h + 1]
                )
                nc.sync.dma_start(
                    out_d[(2 * p + h) * HS : (2 * p + h + 1) * HS, ig * 512 : (ig + 1) * 512],
                    ot,
                )

    qk_tiles = [qk_pool.tile([128, 2, T], bf16, tag="qkt", name=f"qkt{_p}") for _p in range(PAIRS)]
    emit_qk_proj(0, qk_tiles[0])
    emit_v_proj(0)
    for p in range(PAIRS):
        if p + 1 < PAIRS:
            emit_qk_proj(p + 1, qk_tiles[p + 1])
            emit_v_proj(p + 1)
        emit_attention(p, qk_tiles[p])
    ctx.close()


def _build():
    import concourse.tile as tile
    from concourse import bacc, mybir

    f32 = mybir.dt.float32
    nc = bacc.Bacc("TRN2", target_bir_lowering=False, debug=False, num_devices=8)
    x_d = nc.dram_tensor("x", [D, T], mybir.dt.bfloat16, kind="ExternalInput").ap()
    w_d = nc.dram_tensor("w", [D, WCOLS], mybir.dt.bfloat16, kind="ExternalInput").ap()
    bqk_d = nc.dram_tensor("bqk", [128, PAIRS, 2], f32, kind="ExternalInput").ap()
    bv_d = nc.dram_tensor("bv", [VCOLS], f32, kind="ExternalInput").ap()
    out_d = nc.dram_tensor("out", [VCOLS, T], f32, kind="ExternalOutput").ap()
    with tile.TileContext(nc) as tc:
        _emit(tc, x_d, w_d, bqk_d, bv_d, out_d)
    nc.compile()
    return nc


def get_nc():
    if "nc" not in _CACHE:
        _CACHE["nc"] = _build()
    return _CACHE["nc"]


def make_in_maps(x, W_qkv, b_qkv):
    """Shard full inputs into 8 per-core input maps."""
    x = np.asarray(x, dtype=np.float32)
    W_qkv = np.asarray(W_qkv, dtype=np.float32)
    b_qkv = np.asarray(b_qkv, dtype=np.float32)
    in_maps = []
    for c in range(8):
        b, half = divmod(c, 2)
        w_c = W_qkv[:, half * WCOLS : (half + 1) * WCOLS]
        b_c = b_qkv[half * WCOLS : (half + 1) * WCOLS]
        # permute columns: paired q/k blocks first, then v cols in head order
        w3 = w_c.reshape(D, NH_LOCAL, 3, HS)
        blocks = []
        for p in range(PAIRS):
            for qk in range(2):
                blocks.append(w3[:, 2 * p, qk, :])
                blocks.append(w3[:, 2 * p + 1, qk, :])
        for h in range(NH_LOCAL):
            blocks.append(w3[:, h, 2, :])
        w_c = np.concatenate(blocks, axis=1).astype(ml_dtypes.bfloat16)
        # per-partition qk bias: partitions 0-63 <- head 2p, 64-127 <- head 2p+1
        bqk = np.zeros((128, PAIRS, 2), dtype=np.float32)
        for p in range(PAIRS):
            for qk in range(2):
                bqk[0:64, p, qk] = b_c[(2 * p) * 192 + qk * 64 : (2 * p) * 192 + (qk + 1) * 64]
                bqk[64:128, p, qk] = b_c[(2 * p + 1) * 192 + qk * 64 : (2 * p + 1) * 192 + (qk + 1) * 64]
        bv = np.ascontiguousarray(
            b_c.reshape(NH_LOCAL, 3, HS)[:, 2, :].reshape(VCOLS)
        )
        in_maps.append(
            {
                "x": np.ascontiguousarray(x[b].T).astype(ml_dtypes.bfloat16),
                "w": w_c,
                "bqk": bqk,
                "bv": bv,
            }
        )
    return in_maps


def assemble_output(results):
    out = np.zeros((4, T, D), dtype=np.float32)
    for c in range(8):
        b, half = divmod(c, 2)
        out[b, :, half * VCOLS : (half + 1) * VCOLS] = results[c]["out"].T
    return out


def kernel(x, W_qkv, b_qkv):
    from concourse.bass_utils import run_bass_kernel_spmd

    nc = get_nc()
    in_maps = make_in_maps(x, W_qkv, b_qkv)
    res = run_bass_kernel_spmd(nc, in_maps, core_ids=list(range(8)))
    return assemble_output(res.results)


if __name__ == "__main__":
    xs = np.random.randn(4, T, D).astype(np.float32)
    Ws = (np.random.randn(D, 3 * D) / 32.0).astype(np.float32)
    bs = (np.random.randn(3 * D) * 0.02).astype(np.float32)
    o = kernel(xs, Ws, bs)
    print(o.shape, o.dtype)


# revision 20
# speedup vs baseline: 1.0902x; 1.0205x over previous
"""Fused QKV + multi-head attention kernel for Trainium2 (Bass/Tile), 8-core SPMD.

Problem: x[4, 2048, 1024] -> qkv = x @ W_qkv + b_qkv -> 16-head attention -> out[4, 2048, 1024].

Sharding (DP x TP): core c handles batch c//2 and head-group c%2 (8 of 16 heads),
so each core runs the qkv projection for its batch restricted to its heads'
columns of W_qkv, plus full attention for its 8 heads. No cross-core comm.

Per-core kernel design (bf16 matmuls, fp32 accumulation):
 - the host pre-transposes x during sharding, so the device loads xT[k, tok]
   directly with a casting DMA (f32->bf16); no on-device transposes anywhere.
 - qk projection computes qkv^T directly: lhsT = W tile (layout [k, feat] as
   stored), rhs = xT. Heads are processed in pairs: head 2p lives in SBUF
   partitions 0-63 and head 2p+1 in partitions 64-127 (both q^T and k^T), which
   makes the K=64 score matmuls legal (lhsT/rhs share a base partition) and
   lets the PE run the two heads on independent 64-row array tiles.
 - v projection computes v in [tok, feat] orientation (lhsT = xT stationary,
   rhs = W v-columns), with a ones-column appended per head so the attention
   A@V matmul also produces the softmax denominator.
 - scores^T = k^T.T @ q^T accumulate in PSUM; exp (with the 1/8 scale folded
   into the ACT affine) reads [128, 2048] PSUM spans to amortize ACT overhead;
   no max-subtraction (scores ~ N(0,1), exp never overflows fp32/bf16).
 - out^T[65, 512] = [v | 1].T @ E accumulates over key tiles; the denominator
   row is bounced through DRAM into a [128, 4] layout for a 128-lane reciprocal,
   broadcast back across partitions with a stride-0 DMA, and the normalized
   out^T (+ v-bias, added after normalization -- exact) is stored transposed;
   the host un-transposes during unsharding.
"""

import sys

sys.path.insert(0, "/opt/trn_rl_repo")

import numpy as np
import ml_dtypes

T = 2048
D = 1024
NH_LOCAL = 8  # heads per core
HS = 64
WCOLS = NH_LOCAL * 3 * HS  # 1536
VCOLS = NH_LOCAL * HS  # 512
KT = D // 128  # 8 contraction tiles
PAIRS = NH_LOCAL // 2  # 4
IG = T // 512  # 4 query groups
JT = T // 128  # 16 key tiles

_CACHE: dict = {}


def _emit(tc, x_d, w_d, bqk_d, bv_d, out_d):
    import concourse.bass as bass
    from concourse import mybir
    from contextlib import ExitStack

    nc = tc.nc
    f32 = mybir.dt.float32
    bf16 = mybir.dt.bfloat16
    Exp = mybir.ActivationFunctionType.Exp

    ctx = ExitStack()
    pers = ctx.enter_context(tc.tile_pool(name="pers", bufs=1))
    qk_pool = ctx.enter_context(tc.tile_pool(name="qk", bufs=PAIRS))
    e_pool = ctx.enter_context(tc.tile_pool(name="epool", bufs=3))
    o_pool = ctx.enter_context(tc.tile_pool(name="opool", bufs=3))
    sm_pool = ctx.enter_context(tc.tile_pool(name="smpool", bufs=4))
    ot_pool = ctx.enter_context(tc.tile_pool(name="otpool", bufs=4))
    dr_pool = ctx.enter_context(tc.tile_pool(name="drpool", bufs=4, space="DRAM"))
    # PSUM: 8 banks total = ps 2 (proj) + pss 2x2 (scores/exp staging) + po 2 (AV accum).
    # pss double-buffering is what keeps the scores->exp->AV pipeline from
    # serializing (a single buffer stalls the PE ~2.7us per exp group, which
    # also re-throttles the HAM clock gate to 1.2 GHz).
    ps_pool = ctx.enter_context(tc.tile_pool(name="pspool", bufs=2, space="PSUM"))
    pss_pool = ctx.enter_context(tc.tile_pool(name="psspool", bufs=2, space="PSUM"))
    po_pool = ctx.enter_context(tc.tile_pool(name="popool", bufs=2, space="PSUM"))

    # ---- load xT (host pre-transposes and pre-casts to bf16) ----
    xt_sb = pers.tile([128, KT, T], bf16)  # xT[k, tok] per k-tile
    for kk in range(KT):
        nc.sync.dma_start(xt_sb[:, kk, :], x_d[kk * 128 : (kk + 1) * 128, :])

    # ---- constants ----
    bqk_sb = pers.tile([128, PAIRS, 2], f32)  # [part, pair, q/k] per-partition bias
    nc.sync.dma_start(bqk_sb, bqk_d)
    bv_pp = pers.tile([64, NH_LOCAL], f32)  # v-bias, per-partition layout [d, head]
    nc.sync.dma_start(bv_pp, bv_d.rearrange("(h d) -> d h", d=HS))

    # ---- load W (host pre-casts to bf16); pair-0 qk columns first ----
    w_sb = pers.tile([128, KT, WCOLS], bf16)
    for kk in range(KT):
        nc.sync.dma_start(w_sb[:, kk, 0:256], w_d[kk * 128 : (kk + 1) * 128, 0:256])
    for kk in range(KT):
        nc.sync.dma_start(
            w_sb[:, kk, 256:WCOLS], w_d[kk * 128 : (kk + 1) * 128, 256:WCOLS]
        )

    # ---- v ones-columns ----
    v_sb = pers.tile([128, JT, NH_LOCAL, HS + 1], bf16)
    nc.vector.memset(v_sb[:, :, :, HS : HS + 1], 1.0)

    # w is host-permuted: cols [(p*2+qk)*128 : +128] = paired q/k block for pair p,
    # cols [1024:1536] = v columns in head order (walrus requires single-free-dim
    # matmul operand APs, so the pairing permutation happens host-side).
    def emit_qk_proj(p, qk_t):
        # psum partitions 0-63 <- head 2p, 64-127 <- head 2p+1
        for g in range(IG):
            for qk in range(2):
                ps = ps_pool.tile([128, 512], f32, tag="ps")
                c0 = (p * 2 + qk) * 128
                for kk in range(KT):
                    nc.tensor.matmul(
                        ps,
                        w_sb[:, kk, c0 : c0 + 128],
                        xt_sb[:, kk, g * 512 : (g + 1) * 512],
                        start=(kk == 0),
                        stop=(kk == KT - 1),
                    )
                nc.vector.tensor_scalar_add(
                    qk_t[:, qk, g * 512 : (g + 1) * 512], ps, bqk_sb[:, p, qk : qk + 1]
                )

    def emit_v_proj():
        for tt in range(JT):
            ps = ps_pool.tile([128, 512], f32, tag="ps")
            for kk in range(KT):
                nc.tensor.matmul(
                    ps,
                    xt_sb[:, kk, tt * 128 : (tt + 1) * 128],
                    w_sb[:, kk, 1024:1536],
                    start=(kk == 0),
                    stop=(kk == KT - 1),
                )
            nc.vector.tensor_copy(
                v_sb[:, tt, :, 0:HS], ps.rearrange("p (h c) -> p h c", c=HS)
            )

    def emit_attention(p, qk_t):
        for ig in range(IG):
            po = [po_pool.tile([65, 512], f32, tag="po", name=f"po{_h}") for _h in range(2)]
            for jj in range(JT):
                ps = pss_pool.tile([128, 1024], f32, tag="pss")
                for h in range(2):
                    base = 64 * h
                    nc.tensor.matmul(
                        ps[:, h * 512 : (h + 1) * 512],
                        qk_t[base : base + 64, 1, jj * 128 : (jj + 1) * 128],
                        qk_t[base : base + 64, 0, ig * 512 : (ig + 1) * 512],
                        start=True,
                        stop=True,
                    )
                e_t = e_pool.tile([128, 1024], bf16, tag="e")
                nc.scalar.activation(e_t, ps, Exp, scale=0.125)
                for h in range(2):
                    nc.tensor.matmul(
                        po[h],
                        v_sb[:, jj, 2 * p + h, :],
                        e_t[:, h * 512 : (h + 1) * 512],
                        start=(jj == 0),
                        stop=(jj == JT - 1),
                    )
            # normalize: out^T[d, i] = po[d, i] * (1/den[i]) + bv[d], written as out^T.
            # den row sits on partition 64; the reciprocal is computed across 128
            # lanes by bouncing the row through DRAM into a [128, 4] layout, and
            # broadcast back across partitions with a stride-0 DMA.
            o_t = [o_pool.tile([65, 512], f32, tag="o", name=f"ot{_h}") for _h in range(2)]
            rcd_t = dr_pool.tile([2, 512], f32, tag="rcd")
            rct = sm_pool.tile([128, 8], f32, tag="rct")
            for h in range(2):
                nc.vector.tensor_copy(o_t[h], po[h])
                nc.sync.dma_start(rct[:, h * 4 : (h + 1) * 4], o_t[h][64:65, :])
            rcp = sm_pool.tile([128, 8], f32, tag="rcp")
            nc.vector.reciprocal(rcp, rct)
            for h in range(2):
                nc.sync.dma_start(rcd_t[h], rcp[:, h * 4 : (h + 1) * 4])
            for h in range(2):
                den_bc = sm_pool.tile([64, 512], f32, tag="denbc", name=f"dbc{h}")
                rcd_h = rcd_t[h]
                bc_src = bass.AP(
                    tensor=rcd_h.tensor,
                    offset=rcd_h.offset,
                    ap=[[0, 64]] + list(rcd_h.ap),
                )
                nc.gpsimd.dma_start(den_bc, bc_src)
                ot = ot_pool.tile([64, 512], f32, tag="ot")
                nc.vector.tensor_mul(ot, o_t[h][0:64, :], den_bc)
                nc.vector.tensor_scalar_add(
                    ot, ot, bv_pp[:, 2 * p + h : 2 * p + h + 1]
                )
                nc.sync.dma_start(
                    out_d[(2 * p + h) * HS : (2 * p + h + 1) * HS, ig * 512 : (ig + 1) * 512],
                    ot,
                )

    qk_tiles = [qk_pool.tile([128, 2, T], bf16, tag="qkt", name=f"qkt{_p}") for _p in range(PAIRS)]
    emit_qk_proj(0, qk_tiles[0])
    emit_v_proj()
    for p in range(PAIRS):
        if p + 1 < PAIRS:
            emit_qk_proj(p + 1, qk_tiles[p + 1])
        emit_attention(p, qk_tiles[p])
    ctx.close()


def _build():
    import concourse.tile as tile
    from concourse import bacc, mybir

    f32 = mybir.dt.float32
    nc = bacc.Bacc("TRN2", target_bir_lowering=False, debug=False, num_devices=8)
    x_d = nc.dram_tensor("x", [D, T], mybir.dt.bfloat16, kind="ExternalInput").ap()
    w_d = nc.dram_tensor("w", [D, WCOLS], mybir.dt.bfloat16, kind="ExternalInput").ap()
    bqk_d = nc.dram_tensor("bqk", [128, PAIRS, 2], f32, kind="ExternalInput").ap()
    bv_d = nc.dram_tensor("bv", [VCOLS], f32, kind="ExternalInput").ap()
    out_d = nc.dram_tensor("out", [VCOLS, T], f32, kind="ExternalOutput").ap()
    with tile.TileContext(nc) as tc:
        _emit(tc, x_d, w_d, bqk_d, bv_d, out_d)
    nc.compile()
    return nc


def get_nc():
    if "nc" not in _CACHE:
        _CACHE["nc"] = _build()
    return _CACHE["nc"]


def make_in_maps(x, W_qkv, b_qkv):
    """Shard full inputs into 8 per-core input maps."""
    x = np.asarray(x, dtype=np.float32)
    W_qkv = np.asarray(W_qkv, dtype=np.float32)
    b_qkv = np.asarray(b_qkv, dtype=np.float32)
    in_maps = []
    for c in range(8):
        b, half = divmod(c, 2)
        w_c = W_qkv[:, half * WCOLS : (half + 1) * WCOLS]
        b_c = b_qkv[half * WCOLS : (half + 1) * WCOLS]
        # permute columns: paired q/k blocks first, then v cols in head order
        w3 = w_c.reshape(D, NH_LOCAL, 3, HS)
        blocks = []
        for p in range(PAIRS):
            for qk in range(2):
                blocks.append(w3[:, 2 * p, qk, :])
                blocks.append(w3[:, 2 * p + 1, qk, :])
        for h in range(NH_LOCAL):
            blocks.append(w3[:, h, 2, :])
        w_c = np.concatenate(blocks, axis=1).astype(ml_dtypes.bfloat16)
        # per-partition qk bias: partitions 0-63 <- head 2p, 64-127 <- head 2p+1
        bqk = np.zeros((128, PAIRS, 2), dtype=np.float32)
        for p in range(PAIRS):
            for qk in range(2):
                bqk[0:64, p, qk] = b_c[(2 * p) * 192 + qk * 64 : (2 * p) * 192 + (qk + 1) * 64]
                bqk[64:128, p, qk] = b_c[(2 * p + 1) * 192 + qk * 64 : (2 * p + 1) * 192 + (qk + 1) * 64]
        bv = np.ascontiguousarray(
            b_c.reshape(NH_LOCAL, 3, HS)[:, 2, :].reshape(VCOLS)
        )
        in_maps.append(
            {
                "x": np.ascontiguousarray(x[b].T).astype(ml_dtypes.bfloat16),
                "w": w_c,
                "bqk": bqk,
                "bv": bv,
            }
        )
    return in_maps


def assemble_output(results):
    out = np.zeros((4, T, D), dtype=np.float32)
    for c in range(8):
        b, half = divmod(c, 2)
        out[b, :, half * VCOLS : (half + 1) * VCOLS] = results[c]["out"].T
    return out


def kernel(x, W_qkv, b_qkv):
    from concourse.bass_utils import run_bass_kernel_spmd

    nc = get_nc()
    in_maps = make_in_maps(x, W_qkv, b_qkv)
    res = run_bass_kernel_spmd(nc, in_maps, core_ids=list(range(8)))
    return assemble_output(res.results)


if __name__ == "__main__":
    xs = np.random.randn(4, T, D).astype(np.float32)
    Ws = (np.random.randn(D, 3 * D) / 32.0).astype(np.float32)
    bs = (np.random.randn(3 * D) * 0.02).astype(np.float32)
    o = kernel(xs, Ws, bs)
    print(o.shape, o.dtype)


# revision 22
# speedup vs baseline: 1.0999x; 1.0090x over previous
"""Fused QKV + multi-head attention kernel for Trainium2 (Bass/Tile), 8-core SPMD.

Problem: x[4, 2048, 1024] -> qkv = x @ W_qkv + b_qkv -> 16-head attention -> out[4, 2048, 1024].

Sharding (DP x TP): core c handles batch c//2 and head-group c%2 (8 of 16 heads),
so each core runs the qkv projection for its batch restricted to its heads'
columns of W_qkv, plus full attention for its 8 heads. No cross-core comm.

Per-core kernel design (bf16 matmuls, fp32 accumulation):
 - the host pre-transposes x during sharding, so the device loads xT[k, tok]
   directly with a casting DMA (f32->bf16); no on-device transposes anywhere.
 - qk projection computes qkv^T directly: lhsT = W tile (layout [k, feat] as
   stored), rhs = xT. Heads are processed in pairs: head 2p lives in SBUF
   partitions 0-63 and head 2p+1 in partitions 64-127 (both q^T and k^T), which
   makes the K=64 score matmuls legal (lhsT/rhs share a base partition) and
   lets the PE run the two heads on independent 64-row array tiles.
 - v projection computes v in [tok, feat] orientation (lhsT = xT stationary,
   rhs = W v-columns), with a ones-column appended per head so the attention
   A@V matmul also produces the softmax denominator.
 - scores^T = k^T.T @ q^T accumulate in PSUM; exp (with the 1/8 scale folded
   into the ACT affine) reads [128, 2048] PSUM spans to amortize ACT overhead;
   no max-subtraction (scores ~ N(0,1), exp never overflows fp32/bf16).
 - out^T[65, 512] = [v | 1].T @ E accumulates over key tiles; the denominator
   row is bounced through DRAM into a [128, 4] layout for a 128-lane reciprocal,
   broadcast back across partitions with a stride-0 DMA, and the normalized
   out^T (+ v-bias, added after normalization -- exact) is stored transposed;
   the host un-transposes during unsharding.
"""

import sys

sys.path.insert(0, "/opt/trn_rl_repo")

import numpy as np
import ml_dtypes

T = 2048
D = 1024
NH_LOCAL = 8  # heads per core
HS = 64
WCOLS = NH_LOCAL * 3 * HS  # 1536
VCOLS = NH_LOCAL * HS  # 512
KT = D // 128  # 8 contraction tiles
PAIRS = NH_LOCAL // 2  # 4
IG = T // 512  # 4 query groups
JT = T // 128  # 16 key tiles

_CACHE: dict = {}


def _emit(tc, x_d, w_d, bqk_d, bv_d, out_d):
    import concourse.bass as bass
    from concourse import mybir
    from contextlib import ExitStack

    nc = tc.nc
    f32 = mybir.dt.float32
    bf16 = mybir.dt.bfloat16
    Exp = mybir.ActivationFunctionType.Exp

    ctx = ExitStack()
    pers = ctx.enter_context(tc.tile_pool(name="pers", bufs=1))
    qk_pool = ctx.enter_context(tc.tile_pool(name="qk", bufs=PAIRS))
    e_pool = ctx.enter_context(tc.tile_pool(name="epool", bufs=3))
    o_pool = ctx.enter_context(tc.tile_pool(name="opool", bufs=3))
    sm_pool = ctx.enter_context(tc.tile_pool(name="smpool", bufs=4))
    ot_pool = ctx.enter_context(tc.tile_pool(name="otpool", bufs=4))
    dr_pool = ctx.enter_context(tc.tile_pool(name="drpool", bufs=4, space="DRAM"))
    # PSUM: 8 banks total = ps 2 (proj) + pss 2x2 (scores/exp staging) + po 2 (AV accum).
    # pss double-buffering is what keeps the scores->exp->AV pipeline from
    # serializing (a single buffer stalls the PE ~2.7us per exp group, which
    # also re-throttles the HAM clock gate to 1.2 GHz).
    ps_pool = ctx.enter_context(tc.tile_pool(name="pspool", bufs=2, space="PSUM"))
    pss_pool = ctx.enter_context(tc.tile_pool(name="psspool", bufs=2, space="PSUM"))
    po_pool = ctx.enter_context(tc.tile_pool(name="popool", bufs=2, space="PSUM"))

    # ---- load xT (host pre-transposes and pre-casts to bf16) ----
    xt_sb = pers.tile([128, KT, T], bf16)  # xT[k, tok] per k-tile
    for kk in range(KT):
        nc.sync.dma_start(xt_sb[:, kk, :], x_d[kk * 128 : (kk + 1) * 128, :])

    # ---- constants ----
    bqk_sb = pers.tile([128, PAIRS, 2], f32)  # [part, pair, q/k] per-partition bias
    nc.sync.dma_start(bqk_sb, bqk_d)
    bv_pp = pers.tile([64, NH_LOCAL], f32)  # v-bias, per-partition layout [d, head]
    nc.sync.dma_start(bv_pp, bv_d.rearrange("(h d) -> d h", d=HS))

    # ---- load W (host pre-casts to bf16); pair-0 qk columns first ----
    w_sb = pers.tile([128, KT, WCOLS], bf16)
    for kk in range(KT):
        nc.sync.dma_start(w_sb[:, kk, 0:256], w_d[kk * 128 : (kk + 1) * 128, 0:256])
    for kk in range(KT):
        nc.sync.dma_start(
            w_sb[:, kk, 256:WCOLS], w_d[kk * 128 : (kk + 1) * 128, 256:WCOLS]
        )

    # ---- v ones-columns ----
    v_sb = pers.tile([128, JT, NH_LOCAL, HS + 1], bf16)
    nc.vector.memset(v_sb[:, :, :, HS : HS + 1], 1.0)

    # w is host-permuted: cols [(p*2+qk)*128 : +128] = paired q/k block for pair p,
    # cols [1024:1536] = v columns in head order (walrus requires single-free-dim
    # matmul operand APs, so the pairing permutation happens host-side).
    def emit_qk_proj(p, qk_t):
        # psum partitions 0-63 <- head 2p, 64-127 <- head 2p+1
        for g in range(IG):
            for qk in range(2):
                ps = ps_pool.tile([128, 512], f32, tag="ps")
                c0 = (p * 2 + qk) * 128
                for kk in range(KT):
                    nc.tensor.matmul(
                        ps,
                        w_sb[:, kk, c0 : c0 + 128],
                        xt_sb[:, kk, g * 512 : (g + 1) * 512],
                        start=(kk == 0),
                        stop=(kk == KT - 1),
                    )
                nc.vector.tensor_scalar_add(
                    qk_t[:, qk, g * 512 : (g + 1) * 512], ps, bqk_sb[:, p, qk : qk + 1]
                )

    def emit_v_chain(tt):
        # one v-projection chain: v[tok tile tt, all heads] = xT.T @ Wv
        ps = ps_pool.tile([128, 512], f32, tag="ps", name="psv")
        for kk in range(KT):
            nc.tensor.matmul(
                ps,
                xt_sb[:, kk, tt * 128 : (tt + 1) * 128],
                w_sb[:, kk, 1024:1536],
                start=(kk == 0),
                stop=(kk == KT - 1),
            )
        nc.vector.tensor_copy(
            v_sb[:, tt, :, 0:HS], ps.rearrange("p (h c) -> p h c", c=HS)
        )

    def emit_attention(p, qk_t, v_pending=False):
        # v_pending: the v projection hasn't been emitted yet; weave one v chain
        # into each group of ig 0 (just before the AV that consumes it) so the
        # exp stream starts immediately instead of idling behind the v proj.
        for ig in range(IG):
            po = [po_pool.tile([65, 512], f32, tag="po", name=f"po{_h}") for _h in range(2)]
            for jj in range(JT):
                ps = pss_pool.tile([128, 1024], f32, tag="pss")
                for h in range(2):
                    base = 64 * h
                    nc.tensor.matmul(
                        ps[:, h * 512 : (h + 1) * 512],
                        qk_t[base : base + 64, 1, jj * 128 : (jj + 1) * 128],
                        qk_t[base : base + 64, 0, ig * 512 : (ig + 1) * 512],
                        start=True,
                        stop=True,
                    )
                e_t = e_pool.tile([128, 1024], bf16, tag="e")
                nc.scalar.activation(e_t, ps, Exp, scale=0.125)
                if v_pending and ig == 0:
                    emit_v_chain(jj)
                for h in range(2):
                    nc.tensor.matmul(
                        po[h],
                        v_sb[:, jj, 2 * p + h, :],
                        e_t[:, h * 512 : (h + 1) * 512],
                        start=(jj == 0),
                        stop=(jj == JT - 1),
                    )
            # normalize: out^T[d, i] = po[d, i] * (1/den[i]) + bv[d], written as out^T.
            # den row sits on partition 64; the reciprocal is computed across 128
            # lanes by bouncing the row through DRAM into a [128, 4] layout, and
            # broadcast back across partitions with a stride-0 DMA.
            o_t = [o_pool.tile([65, 512], f32, tag="o", name=f"ot{_h}") for _h in range(2)]
            rcd_t = dr_pool.tile([2, 512], f32, tag="rcd")
            rct = sm_pool.tile([128, 8], f32, tag="rct")
            for h in range(2):
                nc.vector.tensor_copy(o_t[h], po[h])
                nc.sync.dma_start(rct[:, h * 4 : (h + 1) * 4], o_t[h][64:65, :])
            rcp = sm_pool.tile([128, 8], f32, tag="rcp")
            nc.vector.reciprocal(rcp, rct)
            for h in range(2):
                nc.sync.dma_start(rcd_t[h], rcp[:, h * 4 : (h + 1) * 4])
            for h in range(2):
                den_bc = sm_pool.tile([64, 512], f32, tag="denbc", name=f"dbc{h}")
                rcd_h = rcd_t[h]
                bc_src = bass.AP(
                    tensor=rcd_h.tensor,
                    offset=rcd_h.offset,
                    ap=[[0, 64]] + list(rcd_h.ap),
                )
                nc.gpsimd.dma_start(den_bc, bc_src)
                ot = ot_pool.tile([64, 512], f32, tag="ot")
                nc.vector.tensor_mul(ot, o_t[h][0:64, :], den_bc)
                nc.vector.tensor_scalar_add(
                    ot, ot, bv_pp[:, 2 * p + h : 2 * p + h + 1]
                )
                nc.sync.dma_start(
                    out_d[(2 * p + h) * HS : (2 * p + h + 1) * HS, ig * 512 : (ig + 1) * 512],
                    ot,
                )

    qk_tiles = [qk_pool.tile([128, 2, T], bf16, tag="qkt", name=f"qkt{_p}") for _p in range(PAIRS)]
    emit_qk_proj(0, qk_tiles[0])
    for p in range(PAIRS):
        if p + 1 < PAIRS:
            emit_qk_proj(p + 1, qk_tiles[p + 1])
        emit_attention(p, qk_tiles[p], v_pending=(p == 0))
    ctx.close()


def _build():
    import concourse.tile as tile
    from concourse import bacc, mybir

    f32 = mybir.dt.float32
    nc = bacc.Bacc("TRN2", target_bir_lowering=False, debug=False, num_devices=8)
    x_d = nc.dram_tensor("x", [D, T], mybir.dt.bfloat16, kind="ExternalInput").ap()
    w_d = nc.dram_tensor("w", [D, WCOLS], mybir.dt.bfloat16, kind="ExternalInput").ap()
    bqk_d = nc.dram_tensor("bqk", [128, PAIRS, 2], f32, kind="ExternalInput").ap()
    bv_d = nc.dram_tensor("bv", [VCOLS], f32, kind="ExternalInput").ap()
    out_d = nc.dram_tensor("out", [VCOLS, T], f32, kind="ExternalOutput").ap()
    with tile.TileContext(nc) as tc:
        _emit(tc, x_d, w_d, bqk_d, bv_d, out_d)
    nc.compile()
    return nc


def get_nc():
    if "nc" not in _CACHE:
        _CACHE["nc"] = _build()
    return _CACHE["nc"]


def make_in_maps(x, W_qkv, b_qkv):
    """Shard full inputs into 8 per-core input maps."""
    x = np.asarray(x, dtype=np.float32)
    W_qkv = np.asarray(W_qkv, dtype=np.float32)
    b_qkv = np.asarray(b_qkv, dtype=np.float32)
    in_maps = []
    for c in range(8):
        b, half = divmod(c, 2)
        w_c = W_qkv[:, half * WCOLS : (half + 1) * WCOLS]
        b_c = b_qkv[half * WCOLS : (half + 1) * WCOLS]
        # permute columns: paired q/k blocks first, then v cols in head order
        w3 = w_c.reshape(D, NH_LOCAL, 3, HS)
        blocks = []
        for p in range(PAIRS):
            for qk in range(2):
                blocks.append(w3[:, 2 * p, qk, :])
                blocks.append(w3[:, 2 * p + 1, qk, :])
        for h in range(NH_LOCAL):
            blocks.append(w3[:, h, 2, :])
        w_c = np.concatenate(blocks, axis=1).astype(ml_dtypes.bfloat16)
        # per-partition qk bias: partitions 0-63 <- head 2p, 64-127 <- head 2p+1
        bqk = np.zeros((128, PAIRS, 2), dtype=np.float32)
        for p in range(PAIRS):
            for qk in range(2):
                bqk[0:64, p, qk] = b_c[(2 * p) * 192 + qk * 64 : (2 * p) * 192 + (qk + 1) * 64]
                bqk[64:128, p, qk] = b_c[(2 * p + 1) * 192 + qk * 64 : (2 * p + 1) * 192 + (qk + 1) * 64]
        bv = np.ascontiguousarray(
            b_c.reshape(NH_LOCAL, 3, HS)[:, 2, :].reshape(VCOLS)
        )
        in_maps.append(
            {
                "x": np.ascontiguousarray(x[b].T).astype(ml_dtypes.bfloat16),
                "w": w_c,
                "bqk": bqk,
                "bv": bv,
            }
        )
    return in_maps


def assemble_output(results):
    out = np.zeros((4, T, D), dtype=np.float32)
    for c in range(8):
        b, half = divmod(c, 2)
        out[b, :, half * VCOLS : (half + 1) * VCOLS] = results[c]["out"].T
    return out


def kernel(x, W_qkv, b_qkv):
    from concourse.bass_utils import run_bass_kernel_spmd

    nc = get_nc()
    in_maps = make_in_maps(x, W_qkv, b_qkv)
    res = run_bass_kernel_spmd(nc, in_maps, core_ids=list(range(8)))
    return assemble_output(res.results)


if __name__ == "__main__":
    xs = np.random.randn(4, T, D).astype(np.float32)
    Ws = (np.random.randn(D, 3 * D) / 32.0).astype(np.float32)
    bs = (np.random.randn(3 * D) * 0.02).astype(np.float32)
    o = kernel(xs, Ws, bs)
    print(o.shape, o.dtype)
